# revision 27
# baseline (speedup 1.0000x reference)
"""Trainium2 Bass kernel for nn_AdversarialLoss_PDD (pairwise JS-divergence loss).

Math (validated vs reference): with raw logits r = f @ W.T + b,
  S  = softmax(r/4)  (tempered), H_i = sum_c S_ic ln S_ic,
  conf = max softmax(r/2),  pseudo = argmax r,
  JS[i,j] = 0.5*(H_i + H_j) + ln2 - 0.5*(A[i,j] + B[i,j])
  A[i,j] = sum_c S[i,c] * ln(S[i,c]+S[j,c]),  B[i,j] = like A with S[j,c] weights.
For the symmetric ss-mask, sum(0.5*(A+B)) == sum(A), so only A is needed there.

Only same-class pairs can contribute (mask is label equality), so phase 2 is
windowed: per source row, a cw-column window of classmate columns plus a qpad
block of confidence-passing target columns, packed by the host so the SPMD
program is identical on every core.  Phase 1 (logits + softmax stats) splits
the 1024 batch rows 128/core; phase 2 splits the 512 source rows 64/core.
Host does only input layout, mask booleans, and the final masked means.
"""

import math
import numpy as np
from contextlib import ExitStack

import concourse.bass as bass
import concourse.tile as tile
from concourse import bacc, mybir
from concourse.bass_utils import run_bass_kernel_spmd

F32 = mybir.dt.float32
BF16 = mybir.dt.bfloat16
FR = mybir.dt.float32r
U32 = mybir.dt.uint32
AL = mybir.AluOpType
AF = mybir.ActivationFunctionType

NCORES = 8
C = 128            # n classes
K = 2048           # in features
N = 1024           # batch (source+target)
BS = 512           # source rows
RPC = N // NCORES  # phase-1 rows per core
IPC = BS // NCORES # phase-2 source rows per core
KCH = K // 128     # contraction chunks

THRESHOLD = 0.05
LN2 = math.log(2.0)

_cache = {}


def _build_phase1():
    """Per core: raw logits for its 128 rows + softmax stats.

    in:  fT [2048,128] (own f rows, transposed), WT [2048,128], bb [128,128]
    out: out [128,132] = S | sum(S*y) | zt | conf | pseudo(bitcast u32)
    (host finishes H = sum(S*y)/4 - ln(zt); no Ln needed on ACT here, so a
    single warm Exp table covers every activation)
    """
    nc = bacc.Bacc(None, target_bir_lowering=False)
    fT = nc.dram_tensor("fT", [K, RPC], F32, kind="ExternalInput")
    WT = nc.dram_tensor("WT", [K, C], F32, kind="ExternalInput")
    bbi = nc.dram_tensor("bb", [RPC, C], F32, kind="ExternalInput")
    out_o = nc.dram_tensor("out", [RPC, C + 4], F32, kind="ExternalOutput")

    with ExitStack() as ctx:
        tc = ctx.enter_context(tile.TileContext(nc))
        pool = ctx.enter_context(tc.tile_pool(name="main", bufs=1))
        psum = ctx.enter_context(
            tc.tile_pool(name="ps", bufs=1, space=bass.MemorySpace.PSUM))

        # warm the Exp table while DMAs run
        warm = pool.tile([128, 1], F32)
        nc.vector.memset(warm[:], 1.0)
        nc.scalar.activation(warm[:], warm[:], AF.Exp)
        # keep PE ramping during the input DMAs
        wpsum = psum.tile([1, 512], F32, padded_shape=[1, 512])
        wsrc = pool.tile([128, 512], F32)
        nc.vector.memset(wsrc[:], 0.0)
        for _ in range(4):
            nc.tensor.matmul(wpsum[0:1, :], wsrc[:, 0:1], wsrc[:],
                             start=True, stop=True)

        fT_r = fT[:, :].rearrange("(n p) r -> p n r", p=128)
        WT_r = WT[:, :].rearrange("(n p) c -> p n c", p=128)
        bb = pool.tile([128, C], F32)
        nc.gpsimd.dma_start(bb[:], bbi[:, :])
        # first chunks small for an early PE start; rest fat, over 3 queues;
        # separate tiles per DMA so matmul deps are exact, not whole-tensor
        qs = [nc.sync, nc.gpsimd, nc.scalar]
        plan = [(0, 1), (1, 1), (2, 2), (4, 4), (8, 4), (12, 4)]
        fts, wts = [], []
        for d, (st0, ln) in enumerate(plan):
            sl = slice(st0, st0 + ln)
            ftd = pool.tile([128, ln, RPC], F32, name=f"ft{d}")
            wtd = pool.tile([128, ln, C], F32, name=f"wt{d}")
            fts.append(ftd)
            wts.append(wtd)
            qa, qb = qs[d % 3], qs[(d + 1) % 3]
            qa.dma_start(ftd[:], fT_r[:, sl, :])
            qb.dma_start(wtd[:], WT_r[:, sl, :])

        yp = psum.tile([RPC, C], F32)
        n = 0
        for d, (st0, ln) in enumerate(plan):
            for j in range(ln):
                nc.tensor.matmul(yp[:], fts[d][:, j, :], wts[d][:, j, :],
                                 start=(n == 0), stop=(n == KCH - 1))
                n += 1
        y = pool.tile([RPC, C], F32)
        nc.vector.scalar_tensor_tensor(y[:], yp[:], 0.0, bb[:], AL.bypass, AL.add)

        comb = pool.tile([RPC, C + 4], F32)
        et = pool.tile([RPC, C], F32)
        zt = pool.tile([RPC, 1], F32)
        nc.scalar.activation(et[:], y[:], AF.Exp, scale=0.25, accum_out=zt[:])
        e2t = pool.tile([RPC, C], F32)
        z2 = pool.tile([RPC, 1], F32)
        nc.scalar.activation(e2t[:], y[:], AF.Exp, scale=0.5, accum_out=z2[:])
        mx8 = pool.tile([RPC, 8], F32)
        nc.vector.max(mx8[:], y[:])
        cmx = pool.tile([RPC, 1], F32)
        nc.scalar.activation(cmx[:], mx8[:, 0:1], AF.Exp, scale=0.5)

        rz = pool.tile([RPC, 1], F32)
        nc.vector.reciprocal(rz[:], zt[:])
        nc.vector.tensor_scalar_mul(comb[:, 0:C], et[:], rz[:])      # S
        junk = pool.tile([RPC, C], F32)
        nc.vector.scalar_tensor_tensor(junk[:], comb[:, 0:C], 0.0, y[:],
                                       AL.bypass, AL.mult,
                                       accum_out=comb[:, C:C + 1])   # sum S*y
        nc.vector.tensor_copy(comb[:, C + 1:C + 2], zt[:])           # zt
        rz2 = pool.tile([RPC, 1], F32)
        nc.vector.reciprocal(rz2[:], z2[:])
        nc.vector.scalar_tensor_tensor(comb[:, C + 2:C + 3], cmx[:], 0.0,
                                       rz2[:], AL.bypass, AL.mult)   # conf
        pix = pool.tile([RPC, 8], U32)
        nc.vector.max_index(pix[:], mx8[:], y[:])
        nc.vector.tensor_copy(comb[:, C + 3:C + 4].bitcast(U32), pix[:, 0:1])
        nc.sync.dma_start(out_o[:, :], comb[:])
    nc.compile()
    return nc


def _build_phase2(cw, qpad):
    """Windowed pairwise kernel.  Per core, slot i handles one source row;
    its q-columns are packed by the host into stx slot i:
      [cw classmate columns | qpad confidence-passing target columns].
    The masked sums only ever need G = sum_c (S_i+S_j) ln(S_i+S_j) per pair
    (for the symmetric ss mask, sum(A) == sum(G)/2), so per slot-group this
    is one DVE broadcast-add, one Ln, one mult, and one ones-matvec on PE.

    in:  STX [128, 64*(cw+qpad)], BC [128, 64]
    out: G [1, 64*(cw+qpad)]
    """
    SW = cw + qpad
    NG = 4
    SPG = IPC // NG          # 16 slots/group
    GW = SPG * SW
    nc = bacc.Bacc(None, target_bir_lowering=False)
    STX = nc.dram_tensor("STX", [C, IPC * SW], F32, kind="ExternalInput")
    BCt = nc.dram_tensor("BC", [C, IPC], F32, kind="ExternalInput")
    ONEi = nc.dram_tensor("ONE", [C, 1], F32, kind="ExternalInput")
    Go = nc.dram_tensor("G", [1, IPC * SW], F32, kind="ExternalOutput")

    with ExitStack() as ctx:
        tc = ctx.enter_context(tile.TileContext(nc))
        pool = ctx.enter_context(tc.tile_pool(name="main", bufs=1))
        gpool = ctx.enter_context(tc.tile_pool(name="grp", bufs=3))
        psum = ctx.enter_context(
            tc.tile_pool(name="ps", bufs=1, space=bass.MemorySpace.PSUM))

        psGs = [psum.tile([1, GW], F32, name=f"psG{g}", padded_shape=[1, 512])
                for g in range(NG)]
        sbG = pool.tile([1, IPC * SW], F32)
        stxs = []
        for g in range(NG):
            stxg = gpool.tile([C, GW], F32, name=f"stx{g}", bufs=1)
            stxs.append(stxg)
        # group-0 inputs first so its chain starts ASAP; stx3 is issued from
        # the scalar engine right after Ln0 (ACT is otherwise busy)
        nc.sync.dma_start(stxs[0][:], STX[:, 0:GW])
        bc = pool.tile([C, IPC], F32)
        nc.sync.dma_start(bc[:], BCt[:, :])
        ones_f = pool.tile([C, 1], F32)
        nc.sync.dma_start(ones_f[:], ONEi[:, :])
        ones = pool.tile([C, 1], FR)
        nc.vector.tensor_copy(ones[:], ones_f[:])
        for g in (1, 2):
            nc.sync.dma_start(stxs[g][:], STX[:, g * GW:(g + 1) * GW])
        for g in range(NG):
            gsl = slice(g * GW, (g + 1) * GW)
            x3 = stxs[g][:, :].rearrange("p (s w) -> p s w", w=SW)
            bc3 = (bc[:, g * SPG:(g + 1) * SPG]
                   .rearrange("p (s o) -> p s o", o=1)
                   .broadcast_to((C, SPG, SW)))
            ug = gpool.tile([C, GW], F32, name="ug")
            u3 = ug[:, :].rearrange("p (s w) -> p s w", w=SW)
            if g % 2 == 0:
                nc.vector.scalar_tensor_tensor(u3, x3, 0.0, bc3,
                                               AL.bypass, AL.add)
            else:
                nc.gpsimd.tensor_tensor(u3, x3, bc3, AL.add)
            lntg = gpool.tile([C, GW], F32, name="lntg")
            nc.scalar.activation(lntg[:], ug[:], AF.Ln)
            if g == 0:
                nc.scalar.dma_start(stxs[3][:], STX[:, 3 * GW:4 * GW])
            emg = gpool.tile([C, GW], FR, name="emg")
            if g % 2 == 0:
                nc.gpsimd.tensor_tensor(emg[:], ug[:], lntg[:], AL.mult)
            else:
                nc.vector.scalar_tensor_tensor(emg[:], ug[:], 0.0, lntg[:],
                                               AL.bypass, AL.mult)
            nc.tensor.matmul(psGs[g][0:1, :], ones[:], emg[:],
                             start=True, stop=True)
            if g % 2 == 0:
                nc.vector.tensor_copy(sbG[:, gsl], psGs[g][0:1, :])
            else:
                nc.scalar.copy(sbG[:, gsl], psGs[g][0:1, :])
        nc.sync.dma_start(Go[0:1, :], sbG[:])
    nc.compile()
    return nc


def _run(nc, in_maps, **kw):
    return run_bass_kernel_spmd(nc, in_maps, core_ids=list(range(NCORES)), **kw)


def kernel(f, W, b, labels_s, _trace=False, _timings=None):
    f = np.ascontiguousarray(np.asarray(f, dtype=np.float32))
    W = np.ascontiguousarray(np.asarray(W, dtype=np.float32))
    b = np.asarray(b, dtype=np.float32)
    labels = np.asarray(labels_s)

    # ---- phase 1: logits + softmax stats, 128 rows/core ----
    if "p1" not in _cache:
        _cache["p1"] = _build_phase1()
    WT = np.ascontiguousarray(W.T)
    bbc = np.ascontiguousarray(np.broadcast_to(b, (RPC, C)))
    in1 = [{"fT": np.ascontiguousarray(f[c * RPC:(c + 1) * RPC, :].T),
            "WT": WT, "bb": bbc} for c in range(NCORES)]
    r1 = _run(_cache["p1"], in1, trace=_trace)
    if _timings is not None:
        _timings.append(("phase1", r1.exec_time_ns))
    out1 = np.concatenate([r1.results[c]["out"] for c in range(NCORES)], axis=0)
    S = out1[:, 0:C]
    sy = out1[:, C].astype(np.float64)
    zt = out1[:, C + 1].astype(np.float64)
    H = 0.25 * sy - np.log(zt)
    conf = out1[:, C + 2]
    pseudo = np.ascontiguousarray(out1[:, C + 3]).view(np.uint32).astype(np.int64)

    # ---- host: windowed column packing ----
    lab = labels[:BS]
    conf_t = conf[BS:]
    pseudo_t = pseudo[BS:]
    passing = np.nonzero(conf_t >= THRESHOLD)[0]
    npass = len(passing)
    qpad = max(16, ((npass + 15) // 16) * 16)
    classmates = {k: np.nonzero(lab == k)[0] for k in np.unique(lab)}
    maxcls = max(len(v) for v in classmates.values())
    cw = max(16, ((maxcls + 15) // 16) * 16)
    SW = cw + qpad
    ST = S.T  # [128, 1024]

    win_cols = np.zeros((BS, cw), np.int64)   # global col index per slot pos
    win_valid = np.zeros((BS, cw), bool)      # real classmate (incl self)
    for i in range(BS):
        cm = classmates[lab[i]]
        win_cols[i, :len(cm)] = cm
        win_valid[i, :len(cm)] = True
    st_cols = np.zeros(qpad, np.int64)
    st_cols[:npass] = BS + passing
    stx_all = np.empty((C, BS * SW), np.float32)
    for i in range(BS):
        stx_all[:, i * SW:i * SW + cw] = ST[:, win_cols[i]]
        stx_all[:, i * SW + cw:(i + 1) * SW] = ST[:, st_cols]

    # ---- phase 2 ----
    key = ("p2", cw, qpad)
    if key not in _cache:
        _cache[key] = _build_phase2(cw, qpad)
    onecol = np.ones((C, 1), np.float32)
    in2 = [{"STX": np.ascontiguousarray(stx_all[:, c * IPC * SW:(c + 1) * IPC * SW]),
            "BC": np.ascontiguousarray(ST[:, c * IPC:(c + 1) * IPC]),
            "ONE": onecol} for c in range(NCORES)]
    r2 = _run(_cache[key], in2, trace=_trace)
    if _timings is not None:
        _timings.append(("phase2", r2.exec_time_ns))
    G = np.concatenate(
        [r2.results[c]["G"].reshape(IPC, SW) for c in range(NCORES)],
        0).astype(np.float64)

    # ---- host: masked means and final loss ----
    # JS_pair = 0.5*(H_i + H_j) + ln2 - 0.5*G_pair
    mask_ss = win_valid & (win_cols != np.arange(BS)[:, None])
    cnt_sym = mask_ss.sum()
    s_sym = (mask_ss * (0.5 * (H[:BS, None] + H[win_cols]) + LN2
                        - 0.5 * G[:, :cw])).sum()
    loss_ss = (s_sym / cnt_sym) if cnt_sym > 0 else 0.0

    if npass > 0:
        mst = (lab[:, None] == pseudo_t[passing][None, :])
        cnt_st = mst.sum()
        Hj = H[BS + passing]
        s_st = (mst * (0.5 * (H[:BS, None] + Hj[None, :]) + LN2
                       - 0.5 * G[:, cw:cw + npass])).sum()
        loss_st = (s_st / cnt_st) if cnt_st > 0 else 0.0
    else:
        loss_st = 0.0

    loss = np.float32(4.0 * (loss_ss + loss_st))
    return (loss, np.float32(0.0))
